# revision 32
# baseline (speedup 1.0000x reference)
"""Multi-head attention (B=4, N=2048, C=768, H=12, D=64) on 8 TRN2 NeuronCores.

Sharding: core c handles batch c//2 and query rows (c%2)*1024 .. +1024, all
heads. Each core recomputes K/V for its full batch; the host ROTATES x[b].T
per core so this core's query rows are always columns 0:1024 (attention is
permutation-invariant over keys, so the SPMD graph stays core-independent).

v6: fully interleaved emission (bf16 throughout).
- A deterministic scheduler interleaves qkv projection chains, score tiles
  (2 matmuls -> [128,1024] PSUM -> exp on ScalarE), and AV-chain units so the
  Scalar engine streams exp from ~t=15us instead of idling through stage 1.
  Chain work is spread across the whole score stream on a credit system so
  the PE always has filler during exp latency windows.
- AV accumulates per head into [65,1024] PSUM over all 16 key tiles; the ones
  column packed into vo gives the softmax denominator in row 64.
- PSUM: scores/chains share a 2x[128,1024] ring (4 banks) + AV 2x[128,1024]
  (4 banks) = 8.
(fp8 DoubleRow scores were tried and reverted: on this hardware a DoubleRow
matmul issues at ~320ns vs bf16's ~259ns for the same 512-col output, i.e.
the 0.5 cycles/row of the cost model does not materialize.)
The 1/sqrt(D) scale is folded into wq on the host.
"""

from collections import deque
from contextlib import ExitStack

import ml_dtypes
import numpy as np

import concourse.bass as bass
import concourse.tile as tile
from concourse import bacc, mybir
from concourse import bass_utils

B, N, C, H, Dh = 4, 2048, 768, 12, 64
P = 128
NCORES = 8
ROWS = N // 2  # query rows per core
SCALE = Dh ** -0.5

BF16 = mybir.dt.bfloat16
F32 = mybir.dt.float32

CB = C // P   # 6 contraction/feature bands
NT = N // P   # 16 key tiles
QC = 2        # query 512-col halves

_cached_nc = None
LAST_RESULT = None  # BassKernelResults of the most recent run (for test harness)


def _build_nc():
    nc = bacc.Bacc(
        "TRN2",
        target_bir_lowering=False,
        debug=False,
        enable_asserts=False,
        num_devices=NCORES,
    )
    xqT_d = nc.dram_tensor("xqT", [C, ROWS], BF16, kind="ExternalInput")
    xkT_d = nc.dram_tensor("xkT", [C, ROWS], BF16, kind="ExternalInput")
    wqT_d = nc.dram_tensor("wqT", [C, C], BF16, kind="ExternalInput")
    wkT_d = nc.dram_tensor("wkT", [C, C], BF16, kind="ExternalInput")
    wvT_d = nc.dram_tensor("wvT", [C, C], BF16, kind="ExternalInput")
    wprojT_d = nc.dram_tensor("wprojT", [C, C], BF16, kind="ExternalInput")
    bproj_d = nc.dram_tensor("bproj", [CB, P, 1], F32, kind="ExternalInput")
    out_d = nc.dram_tensor("out", [C, ROWS], F32, kind="ExternalOutput")

    Exp = mybir.ActivationFunctionType.Exp

    with tile.TileContext(nc) as tc:
        with ExitStack() as ctx:
            # ---- persistent SBUF pools ----
            pool_wp = ctx.enter_context(tc.tile_pool(name="wp", bufs=1))
            pool_bias = ctx.enter_context(tc.tile_pool(name="bias", bufs=1))
            pool_qT = ctx.enter_context(tc.tile_pool(name="qT", bufs=1))
            pool_kT = ctx.enter_context(tc.tile_pool(name="kT", bufs=1))
            pool_vo = ctx.enter_context(tc.tile_pool(name="vo", bufs=1))
            pool_attT = ctx.enter_context(tc.tile_pool(name="attT", bufs=1))
            pool_x = ctx.enter_context(tc.tile_pool(name="x", bufs=1))
            pool_w = ctx.enter_context(tc.tile_pool(name="w", bufs=1))
            pool_u = ctx.enter_context(tc.tile_pool(name="u", bufs=8))
            pool_r = ctx.enter_context(tc.tile_pool(name="r", bufs=4))
            pool_rb = ctx.enter_context(tc.tile_pool(name="rb", bufs=4))
            pool_y = ctx.enter_context(tc.tile_pool(name="y", bufs=3))
            # ---- PSUM: 4 + 4 = 8 banks. X chains share the sc ring. ----
            ps_sc = ctx.enter_context(tc.tile_pool(name="ps_sc", bufs=2, space="PSUM"))
            ps_u = ctx.enter_context(tc.tile_pool(name="ps_u", bufs=2, space="PSUM"))

            wp_sb = [pool_wp.tile([P, C], BF16, name=f"wp{i}") for i in range(CB)]
            bias_sb = [pool_bias.tile([P, 1], F32, name=f"bias{i}") for i in range(CB)]
            qT_sb = [
                [pool_qT.tile([P, 512], BF16, name=f"qT{ob}_{qc}") for qc in range(QC)]
                for ob in range(CB)
            ]
            kT_sb = [
                [pool_kT.tile([P, 512], BF16, name=f"kT{ob}_{c}") for c in range(4)]
                for ob in range(CB)
            ]
            vo_sb = [pool_vo.tile([P, H * (Dh + 1)], BF16, name=f"vo{i}") for i in range(NT)]
            attT_sb = [pool_attT.tile([P, ROWS], BF16, name=f"attT{i}") for i in range(CB)]
            xq_sb = [pool_x.tile([P, ROWS], BF16, name=f"xq{i}") for i in range(CB)]
            xk_sb = [pool_x.tile([P, ROWS], BF16, name=f"xk{i}") for i in range(CB)]
            wq_sb = [pool_w.tile([P, C], BF16, name=f"wq{i}") for i in range(CB)]
            wk_sb = [pool_w.tile([P, C], BF16, name=f"wk{i}") for i in range(CB)]
            wv_sb = [pool_w.tile([P, C], BF16, name=f"wv{i}") for i in range(CB)]

            # ---- DMAs in priority order ----
            for cb in range(CB):
                nc.sync.dma_start(wq_sb[cb][:], wqT_d.ap()[cb * P:(cb + 1) * P, :])
                nc.sync.dma_start(xq_sb[cb][:], xqT_d.ap()[cb * P:(cb + 1) * P, :])
            for cb in range(CB):
                nc.sync.dma_start(wk_sb[cb][:], wkT_d.ap()[cb * P:(cb + 1) * P, :])
            for cb in range(CB):
                nc.sync.dma_start(xk_sb[cb][:], xkT_d.ap()[cb * P:(cb + 1) * P, :])
                nc.sync.dma_start(wv_sb[cb][:], wvT_d.ap()[cb * P:(cb + 1) * P, :])
            for cb in range(CB):
                nc.sync.dma_start(wp_sb[cb][:], wprojT_d.ap()[cb * P:(cb + 1) * P, :])
                nc.sync.dma_start(bias_sb[cb][:], bproj_d.ap()[cb, :, :])
            for nt in range(NT):
                nc.gpsimd.memset(vo_sb[nt][:], 1.0)

            # ---- unit emitters ----
            def emit_q(ob, qc):
                pt = ps_sc.tile([P, 512], F32, name="px", tag="ps")
                for cb in range(CB):
                    nc.tensor.matmul(
                        pt[:],
                        wq_sb[cb][:, ob * P:(ob + 1) * P],
                        xq_sb[cb][:, qc * 512:(qc + 1) * 512],
                        start=(cb == 0),
                        stop=(cb == CB - 1),
                    )
                nc.vector.tensor_copy(qT_sb[ob][qc][:], pt[:])

            def emit_k(ob, c):  # c in 0..3: key cols c*512 .. +512
                pt = ps_sc.tile([P, 512], F32, name="px", tag="ps")
                src = xq_sb if c < 2 else xk_sb
                off = (c % 2) * 512
                for cb in range(CB):
                    nc.tensor.matmul(
                        pt[:],
                        wk_sb[cb][:, ob * P:(ob + 1) * P],
                        src[cb][:, off:off + 512],
                        start=(cb == 0),
                        stop=(cb == CB - 1),
                    )
                nc.vector.tensor_copy(kT_sb[ob][c][:], pt[:])

            def emit_v(nt, chunk):  # chunk 0: heads 0..7; chunk 1: heads 8..11
                pt = ps_sc.tile([P, 512], F32, name="px", tag="ps")
                width = 512 if chunk == 0 else 256
                src = xq_sb if nt < 8 else xk_sb
                col = (nt % 8) * P
                for cb in range(CB):
                    nc.tensor.matmul(
                        pt[:, 0:width],
                        src[cb][:, col:col + P],
                        wv_sb[cb][:, chunk * 512:chunk * 512 + width],
                        start=(cb == 0),
                        stop=(cb == CB - 1),
                    )
                h0 = chunk * 8
                nh = width // Dh
                nc.vector.tensor_copy(
                    vo_sb[nt].rearrange("p (h e) -> p h e", e=Dh + 1)[:, h0:h0 + nh, 0:Dh],
                    pt[:, 0:width].rearrange("p (h e) -> p h e", e=Dh),
                )

            uts = {}
            pus = {}

            def emit_sc(h, kt):
                band, hp = divmod(h, 2)
                po = hp * 64
                ps = ps_sc.tile([P, 1024], F32, name="ps", tag="ps")
                for qc in range(QC):
                    nc.tensor.matmul(
                        ps[:, qc * 512:(qc + 1) * 512],
                        kT_sb[band][kt // 4][po:po + 64, (kt % 4) * P:(kt % 4 + 1) * P],
                        qT_sb[band][qc][po:po + 64, :],
                        start=True,
                        stop=True,
                    )
                ut = pool_u.tile([P, 1024], BF16, name="ut")
                nc.scalar.activation(ut[:], ps[:], Exp)
                uts[(h, kt)] = ut

            def emit_av(h, kt):
                if kt == 0:
                    pus[h] = ps_u.tile([P, 1024], F32, name="pu", tag="pu")
                pu = pus[h]
                for qc in range(QC):
                    nc.tensor.matmul(
                        pu[0:65, qc * 512:(qc + 1) * 512],
                        vo_sb[kt][:, h * 65:(h + 1) * 65],
                        uts[(h, kt)][:, qc * 512:(qc + 1) * 512],
                        start=(kt == 0),
                        stop=(kt == NT - 1),
                    )
                del uts[(h, kt)]

            def emit_normalize(h):
                band, hp = divmod(h, 2)
                po = hp * 64
                pu = pus.pop(h)
                # halves pipelined so downstream unblocks sooner
                for qc in range(QC):
                    sl = slice(qc * 512, (qc + 1) * 512)
                    s = pool_r.tile([1, 512], F32, name="s", tag="r")
                    nc.vector.tensor_copy(s[:], pu[64:65, sl])
                    r = pool_r.tile([1, 512], F32, name="r", tag="r")
                    nc.vector.reciprocal_approx_fast(r[:], s[:])
                    rb = pool_rb.tile([64, 512], F32, name="rb")
                    nc.gpsimd.partition_broadcast(rb[:], r[:])
                    nc.vector.tensor_mul(
                        attT_sb[band][po:po + 64, sl], pu[0:64, sl], rb[:]
                    )

            def emit_proj(ob, qc):
                pt = ps_sc.tile([P, 512], F32, name="pt_y", tag="ps")
                for cb in range(CB):
                    nc.tensor.matmul(
                        pt[:],
                        wp_sb[cb][:, ob * P:(ob + 1) * P],
                        attT_sb[cb][:, qc * 512:(qc + 1) * 512],
                        start=(cb == 0),
                        stop=(cb == CB - 1),
                    )
                y = pool_y.tile([P, 512], F32, name="y")
                nc.vector.tensor_scalar_add(y[:], pt[:], bias_sb[ob][:])
                nc.sync.dma_start(
                    out_d.ap()[ob * P:(ob + 1) * P, qc * 512:(qc + 1) * 512], y[:]
                )

            # ---- deterministic round-robin interleaved emission ----
            AV_LAG = 2  # av(h,kt) waits until >=2 sc tiles emitted after sc(h,kt)
            emitted = set()
            sc_emit_idx = {}
            n_sc_emitted = 0

            sc_q = deque((h, kt) for h in range(H) for kt in range(NT))
            av_q = deque((h, kt) for h in range(H) for kt in range(NT))

            X = deque()
            X += [("q", 0, 0), ("q", 0, 1), ("k", 0, 0), ("k", 0, 1)]
            X += [("v", 0, 0), ("v", 0, 1), ("v", 1, 0), ("v", 1, 1)]
            X += [("k", 0, 2), ("k", 0, 3)]
            X += [("v", nt, ch) for nt in range(2, 4) for ch in range(2)]
            X += [("q", 1, 0), ("q", 1, 1)] + [("k", 1, c) for c in range(4)]
            X += [("v", nt, ch) for nt in range(4, 8) for ch in range(2)]
            X += [("q", 2, 0), ("q", 2, 1)] + [("k", 2, c) for c in range(4)]
            X += [("v", nt, ch) for nt in range(8, 12) for ch in range(2)]
            X += [("q", 3, 0), ("q", 3, 1)] + [("k", 3, c) for c in range(4)]
            X += [("v", nt, ch) for nt in range(12, 16) for ch in range(2)]
            X += [("q", 4, 0), ("q", 4, 1)] + [("k", 4, c) for c in range(4)]
            X += [("q", 5, 0), ("q", 5, 1)] + [("k", 5, c) for c in range(4)]

            def sc_ready(h, kt):
                band = h // 2
                return (
                    ("q", band, 0) in emitted
                    and ("q", band, 1) in emitted
                    and ("k", band, kt // 4) in emitted
                )

            def av_ready(h, kt):
                return (
                    (h, kt) in sc_emit_idx
                    and n_sc_emitted - sc_emit_idx[(h, kt)] >= AV_LAG
                    and ("v", kt, 0) in emitted
                    and ("v", kt, 1) in emitted
                )

            # sc may run at most UMAX uts tiles ahead of av, else the pool_u
            # ring wedges ACT against PE (deadlock, not just a stall)
            UMAX = 6
            uts_inflight = 0

            def do_sc():
                nonlocal n_sc_emitted, uts_inflight
                h, kt = sc_q.popleft()
                emit_sc(h, kt)
                sc_emit_idx[(h, kt)] = n_sc_emitted
                n_sc_emitted += 1
                uts_inflight += 1

            def do_av():
                nonlocal uts_inflight
                h, kt = av_q.popleft()
                emit_av(h, kt)
                uts_inflight -= 1
                if kt == NT - 1:
                    emit_normalize(h)

            def do_x():
                u = X.popleft()
                if u[0] == "q":
                    emit_q(u[1], u[2])
                elif u[0] == "k":
                    emit_k(u[1], u[2])
                else:
                    emit_v(u[1], u[2])
                emitted.add(u)

            # cycle [sc, sc, av, av, X]; skip kinds whose front isn't ready
            while sc_q or av_q or X:
                progress = False
                for _ in range(2):
                    if sc_q and sc_ready(*sc_q[0]) and uts_inflight < UMAX:
                        do_sc()
                        progress = True
                for _ in range(2):
                    if av_q and av_ready(*av_q[0]):
                        do_av()
                        progress = True
                if X:
                    do_x()
                    progress = True
                if not progress:
                    # only sc/av left, both gated: relax the AV lag
                    h, kt = av_q[0]
                    assert (h, kt) in sc_emit_idx and ("v", kt, 0) in emitted and (
                        "v", kt, 1
                    ) in emitted, ("scheduler deadlock", av_q[0])
                    do_av()

            # ---- output projection tail ----
            for ob in range(CB):
                for qc in range(QC):
                    emit_proj(ob, qc)

    nc.compile()
    return nc


def kernel(x, w_qkv, w_proj, b_proj):
    global _cached_nc, LAST_RESULT
    if _cached_nc is None:
        _cached_nc = _build_nc()
    nc = _cached_nc

    x = np.asarray(x, dtype=np.float32)
    w_qkv = np.asarray(w_qkv, dtype=np.float32)
    w_proj = np.asarray(w_proj, dtype=np.float32)
    b_proj = np.asarray(b_proj, dtype=np.float32)

    bf = ml_dtypes.bfloat16
    wqkvT = w_qkv.T.astype(np.float32).copy()  # [C, 3C]
    wqkvT[:, :C] *= SCALE  # fold q scaling
    wqT = np.ascontiguousarray(wqkvT[:, 0:C]).astype(bf)
    wkT = np.ascontiguousarray(wqkvT[:, C:2 * C]).astype(bf)
    wvT = np.ascontiguousarray(wqkvT[:, 2 * C:3 * C]).astype(bf)
    wprojT = np.ascontiguousarray(w_proj.T).astype(bf)
    bproj_dev = np.ascontiguousarray(b_proj.astype(np.float32).reshape(CB, P, 1))

    in_maps = []
    for c in range(NCORES):
        b, half = divmod(c, 2)
        xTb = x[b].T.astype(bf)  # [C, N]
        if half:
            xTb = np.roll(xTb, -ROWS, axis=1)  # query rows -> columns 0:1024
        in_maps.append(
            {
                "xqT": np.ascontiguousarray(xTb[:, 0:ROWS]),
                "xkT": np.ascontiguousarray(xTb[:, ROWS:N]),
                "wqT": wqT,
                "wkT": wkT,
                "wvT": wvT,
                "wprojT": wprojT,
                "bproj": bproj_dev,
            }
        )

    res = bass_utils.run_bass_kernel_spmd(nc, in_maps, core_ids=list(range(NCORES)))
    LAST_RESULT = res

    out = np.empty((B, N, C), np.float32)
    for c in range(NCORES):
        b, half = divmod(c, 2)
        out[b, half * ROWS:(half + 1) * ROWS, :] = res.results[c]["out"].T
    return out
